# revision 73
# baseline (speedup 1.0000x reference)
"""Trainium2 Bass kernel for nn_BlockDecomposition (relational GNN message passing).

Reference computation:
    out[n] = keep[n] * (x[n] @ BD(blocks[-1]))                    (self loop)
           + sum_{directed edge e: tgt_e == n} w_e * (x[src_e] @ BD(blocks[et_e]))
where BD(.) embeds 32 4x4 blocks into a block-diagonal 128x128 matrix and the
edge list is symmetrized (each undirected edge appears in both directions).

Strategy (8 NeuronCores, no collectives; one SPMD program):
  - Shard by TARGET node. A host-side greedy balancer assigns each node to a
    (core, block of 128, column) bin so per-(block, relation) in-degree
    counts match across cores -- the static schedule is the max over cores,
    so balance = less padding. Each core computes its nodes' output rows
    completely; the host scatters rows back to node order.
  - Per block one dma_gather (GPSIMD SWDGE) pulls all needed x rows from
    the HBM table into SBUF, laid out [edge mod 128 (partition), tile, 128
    features]. The table is bit-packed as int32 pairs (64 x 4B = 256B per
    row, same bytes as 128 fp16): gather descriptor-generation cost scales
    with element count, not bytes, so packing halves Pool engine time. The
    SBUF tile is bitcast back to fp16 for the matmuls.
  - Relations are organized per block into 4 supergroups of 4 relation
    "slots" sharing a [din, 4*128] PSUM bank. Each relation contributes
    floor(gmax/128) dense 128-edge "full" tiles; supergroup remainders are
    concatenated into shared "merged" tiles (one-hot column = 128*slot +
    tloc). The first merged tile spans the whole bank and doubles as the
    PSUM reset (start=True); every other tile accumulates through a tight
    per-tile column window [a0, a1) -- edges are sorted by target column,
    so 128 sorted edges span ~70 columns, and PE matmul cost is
    proportional to OUTPUT width only. Per tile:
      * a weighted one-hot OH[e, col] = (iota[col] == tloc[e]) * w[e] is
        built in one fused tensor_scalar (is_equal, mult) on DVE or Pool,
        or as Abs+Relu on ACT -- engine chosen by an exact min-max planner
        (narrow tiles are cheapest on Pool, wide on DVE) interleaved
        Bresenham-style for temporal uniformity;
      * PE scatter-matmul aggT[din, col] += xg[e, din].T-contract OH[e, col]
        (fp16 x fp16, fp32 PSUM accumulate).
    Per supergroup: one PSUM->SBUF fp16 copy (ACT or DVE; GPSIMD cannot
    touch PSUM on real HW); then per relation a PE transform matmul
    out[n, dout] += agg[n, din] @ BD(W_r)[din, dout] accumulates into the
    block's output PSUM bank. Transforms run one supergroup behind the
    scatters (crossing block borders) so the copy latency never stalls PE.
  - Self-loops bypass the gather/scatter path: a host-transposed masked xT
    slice [din, 128 nodes] is DMA'd per block (contiguous, no gather) and
    used directly as the lhsT of the relation-16 transform.
  - Startup: block 0's gather indices and per-block metadata load first and
    the first gather is chunked so PE starts ~3us in; constants stream on
    SP/ACT queues in parallel.

Numerics: gathered x, one-hots, and block weights are fp16; accumulation is
fp32 in PSUM (end-to-end rel err ~4e-4 vs fp64 reference). All floating-
point arithmetic happens on device. Host work is index manipulation
(balancing/sorting/padding/layout), dtype casts/bit-packing, and placing
weight values into the block-diagonal layout.

Gathers are chunked (~4 tiles per dma_gather) so Pool interleaves one-hot
builds between transfer chunks instead of blocking PE for 1.8us stretches.

Cost-model engine busy per core (runtime ~35.5us): DVE ~27.6us (one-hots +
copies), PE 26.9us (scatter + transform matmuls), ACT ~26.3us (copies),
Pool ~23.3us (gathers + one-hots), SP ~21.7us (DMA queues). PSUM: 6 agg
banks + 2 output banks so scatters run a full supergroup ahead of copies.
"""

import os
import sys
import numpy as np

for _p in ("/opt/trn_rl_repo", "/root/.axon_site/_ro/trn_rl_repo"):
    if os.path.isdir(_p) and _p not in sys.path:
        sys.path.insert(0, _p)

import concourse.bass as bass
import concourse.bacc as bacc
import concourse.mybir as mybir
import concourse.tile as tile
from concourse.bass_utils import run_bass_kernel_spmd

# ----------------------------------------------------------------------------
# Problem constants (hardcoded per spec)
N_NODES = 10000
N_EDGES = 160000
NUM_REL = 16          # relations used by edges; blocks[16] is the self-loop
NUM_BLOCKS = 32
BLOCK_SIZE = 4
D = NUM_BLOCKS * BLOCK_SIZE   # 128
N_CORES = 8
NPC = N_NODES // N_CORES      # 1250 nodes per core
BLK = 128                     # node block size (partition dim of scatter)
NBLK = (NPC + BLK - 1) // BLK  # 10 blocks per core (last one partial: 98)
NRELS = NUM_REL + 1           # 16 edge relations + self-loop transform 16
TILE_E = 128                  # edges per tile (matmul contraction dim)
PACK32 = 64                   # int32 elements per packed x row (= 256B)

F32 = mybir.dt.float32
F16 = mybir.dt.float16
I16 = mybir.dt.int16
I32 = mybir.dt.int32

_DEBUG_SIM = os.environ.get("KERNEL_USE_CORESIM", "0") == "1"


# ----------------------------------------------------------------------------
# Host-side preprocessing: integer index manipulation only.

SUPERGROUPS = [list(range(4 * g, 4 * g + 4)) for g in range(4)]


def _build_schedule(cnt):
    """Static tile schedule shared by all cores.

    cnt: [C, NBLK, NUM_REL] per-core (block, rel) edge counts.

    Returns (sched, Ttot): per block, supergroups of <=4 relation slots with
    their merged + full tile lists. Tile column windows (a0, a1) are filled
    in later by _preprocess from the actual sorted edge data; here they are
    initialized to the full spans. The first merged tile doubles as the
    PSUM-bank reset (start=True over the whole used region); all other
    tiles accumulate through tight windows.
    """
    gmax = cnt.max(axis=0)  # [NBLK, NUM_REL]
    sched = []
    Ttot = 0
    for b in range(NBLK):
        sgs = []
        for rels_all in SUPERGROUPS:
            rels = [r for r in rels_all if gmax[b, r] > 0]
            if not rels:
                continue
            slots = {r: j for j, r in enumerate(rels)}
            full = {r: int(gmax[b, r]) // TILE_E for r in rels}
            rem = {r: int(gmax[b, r]) % TILE_E for r in rels}
            rem_total = sum(rem.values())
            m = (rem_total + TILE_E - 1) // TILE_E
            nslots = len(rels)
            used = nslots * BLK

            tiles = []  # dicts: kind, a0, a1 (bank-absolute cols), start, stop
            for i in range(m):
                tiles.append(
                    {"kind": "merged", "i": i, "a0": 0 if i == 0 else None,
                     "a1": used if i == 0 else None, "start": i == 0}
                )
            for r in rels:
                j = slots[r]
                for t in range(full[r]):
                    tiles.append(
                        {"kind": "full", "r": r, "j": j, "k": t,
                         "a0": j * BLK, "a1": (j + 1) * BLK, "start": False}
                    )
            for t in tiles:
                t["stop"] = False
            tiles[-1]["stop"] = True
            sgs.append(
                {
                    "rels": rels,
                    "slots": slots,
                    "full": full,
                    "rem": rem,
                    "m": m,
                    "used": used,
                    "need_reset": m == 0,
                    "ntiles": len(tiles),
                    "tiles": tiles,
                }
            )
            Ttot += len(tiles)
        sgs.sort(key=lambda s: s["ntiles"])
        sched.append({"sgs": sgs})
    return sched, Ttot


def _balance_nodes(tgtA, etA):
    """Assign nodes to (core, block, tloc) bins so that per-(block, rel)
    in-degree counts are as equal as possible across the 8 cores -- the
    static tile schedule is the max over cores, so imbalance = padding.

    Greedy: nodes in decreasing total degree order; each goes to the bin
    (capacity 128) whose per-rel count increase beyond the current
    cross-core max is smallest.  Pure integer index manipulation.

    Returns pos_core[n], pos_blk[n], pos_tloc[n].
    """
    deg = np.zeros((N_NODES, NUM_REL), dtype=np.int32)
    np.add.at(deg, (tgtA, etA), 1)
    order = np.argsort(-deg.sum(axis=1), kind="stable")
    cnt = np.zeros((N_CORES, NBLK, NUM_REL), dtype=np.int32)
    fill = np.zeros((N_CORES, NBLK), dtype=np.int32)
    pos_core = np.zeros(N_NODES, dtype=np.int64)
    pos_blk = np.zeros(N_NODES, dtype=np.int64)
    pos_tloc = np.zeros(N_NODES, dtype=np.int64)
    for n in order:
        d = deg[n][None, None, :]
        gmax = cnt.max(axis=0)[None, :, :]
        inc = np.maximum(cnt + d - gmax, 0).sum(axis=2).astype(np.float64)
        inc += np.where(fill >= BLK, np.inf, 0.0)
        inc += fill * 1e-4  # tie-break: least-filled bin
        c, b = np.unravel_index(np.argmin(inc), inc.shape)
        pos_core[n], pos_blk[n], pos_tloc[n] = c, b, fill[c, b]
        cnt[c, b] += deg[n]
        fill[c, b] += 1
    return pos_core, pos_blk, pos_tloc


def _preprocess(source, target, edge_type, edge_weights):
    """Build the per-core padded tile schedule (edges only; no self-loops).

    Returns sched, Ttot, node position maps, plus per-core arrays:
      src_pad  [C, Ttot*128] int16   source node id per edge slot
      tloc_pad [C, Ttot*128] float32 one-hot column per edge slot
      w_pad    [C, Ttot*128] float32 edge weight per edge slot (0 for pads)
    """
    src = np.asarray(source).astype(np.int64)
    tgt = np.asarray(target).astype(np.int64)
    et = np.asarray(edge_type).astype(np.int64)
    ew = np.asarray(edge_weights).astype(np.float32)

    # symmetrize: messages flow both directions with same relation/weight
    srcA = np.concatenate([src, tgt])
    tgtA = np.concatenate([tgt, src])
    etA = np.concatenate([et, et])
    ewA = np.concatenate([ew, ew])

    pos_core, pos_blk, pos_tloc = _balance_nodes(tgtA, etA)
    core = pos_core[tgtA]
    blk = pos_blk[tgtA]
    tloc = pos_tloc[tgtA]

    # sort by (core, block, rel, tloc): within each cell edges are ordered by
    # target column, so each 128-edge tile spans a narrow column window.
    order = np.lexsort((tloc, etA, blk, core))
    srcS = srcA[order].astype(np.int16)
    tlocS = tloc[order].astype(np.float32)
    ewS = ewA[order]

    key = (core * NBLK + blk) * NUM_REL + etA
    cnt = np.bincount(key, minlength=N_CORES * NBLK * NUM_REL).reshape(
        N_CORES, NBLK, NUM_REL
    )
    starts = np.concatenate([[0], np.cumsum(cnt.reshape(-1))]).astype(np.int64)

    sched, Ttot = _build_schedule(cnt)

    src_pad = np.zeros((N_CORES, Ttot * TILE_E), dtype=np.int16)
    tloc_pad = np.zeros((N_CORES, Ttot * TILE_E), dtype=np.float32)
    w_pad = np.zeros((N_CORES, Ttot * TILE_E), dtype=np.float32)

    # pass 1: per-tile column windows (union of real edge columns over cores)
    for b in range(NBLK):
        for sg in sched[b]["sgs"]:
            for t in sg["tiles"]:
                if t["kind"] == "full":
                    r, j, k = t["r"], t["j"], t["k"]
                    w0, w1 = BLK, 0
                    for c in range(N_CORES):
                        s0 = int(starts[(c * NBLK + b) * NUM_REL + r])
                        n = int(cnt[c, b, r])
                        lo_r, hi_r = TILE_E * k, min(TILE_E * (k + 1), n)
                        if hi_r <= lo_r:
                            continue
                        w0 = min(w0, int(tlocS[s0 + lo_r]))
                        w1 = max(w1, int(tlocS[s0 + hi_r - 1]) + 1)
                    assert w0 < w1
                    t["a0"], t["a1"] = j * BLK + w0, j * BLK + w1
            # merged tiles i>0: window from the concatenated remainder stream
            if sg["m"] > 1:
                lo = {i: sg["used"] for i in range(1, sg["m"])}
                hi = {i: 0 for i in range(1, sg["m"])}
                for c in range(N_CORES):
                    soff = 0
                    for r in sg["rels"]:
                        s0 = int(starts[(c * NBLK + b) * NUM_REL + r])
                        n = int(cnt[c, b, r])
                        nf = min(n, sg["full"][r] * TILE_E)
                        j = sg["slots"][r]
                        for q in range(n - nf):
                            i = (soff + q) // TILE_E
                            if i == 0:
                                continue
                            col = j * BLK + int(tlocS[s0 + nf + q])
                            lo[i] = min(lo[i], col)
                            hi[i] = max(hi[i], col + 1)
                        soff += sg["rem"][r]
                for t in sg["tiles"]:
                    if t["kind"] == "merged" and t["i"] > 0:
                        i = t["i"]
                        assert lo[i] < hi[i]
                        t["a0"], t["a1"] = lo[i], hi[i]

    # pass 2: fill per-core slot data; pads sit at their tile's window start
    for c in range(N_CORES):
        pos = 0
        for b in range(NBLK):
            for sg in sched[b]["sgs"]:
                # prefill pad columns with each tile's window start
                for ti, t in enumerate(sg["tiles"]):
                    tloc_pad[c, pos + ti * TILE_E : pos + (ti + 1) * TILE_E] = t["a0"]
                mslots = sg["m"] * TILE_E
                fbase = pos + mslots
                moff = pos
                for r in sg["rels"]:
                    gi = (c * NBLK + b) * NUM_REL + r
                    s0 = int(starts[gi])
                    n = int(cnt[c, b, r])
                    j = sg["slots"][r]
                    nfull_slots = sg["full"][r] * TILE_E
                    nf = min(n, nfull_slots)
                    src_pad[c, fbase : fbase + nf] = srcS[s0 : s0 + nf]
                    tloc_pad[c, fbase : fbase + nf] = tlocS[s0 : s0 + nf] + 128.0 * j
                    w_pad[c, fbase : fbase + nf] = ewS[s0 : s0 + nf]
                    fbase += nfull_slots
                    nr = n - nf
                    assert 0 <= nr <= sg["rem"][r]
                    src_pad[c, moff : moff + nr] = srcS[s0 + nf : s0 + n]
                    tloc_pad[c, moff : moff + nr] = tlocS[s0 + nf : s0 + n] + 128.0 * j
                    w_pad[c, moff : moff + nr] = ewS[s0 + nf : s0 + n]
                    moff += sg["rem"][r]
                pos += sg["ntiles"] * TILE_E
        assert pos == Ttot * TILE_E
    return sched, Ttot, (pos_core, pos_blk, pos_tloc), src_pad, tloc_pad, w_pad


def _make_bdw(blocks):
    """blocks [17, 32, 4, 4] -> dense block-diagonal lhsT layout [128, 17*128]
    with BDW[:, r*128:(r+1)*128][4b+i, 4b+j] = blocks[r, b, i, j]."""
    blocks = np.asarray(blocks).astype(np.float32)
    bdw = np.zeros((D, NRELS * D), dtype=np.float32)
    for r in range(NRELS):
        for b in range(NUM_BLOCKS):
            bdw[
                b * BLOCK_SIZE : (b + 1) * BLOCK_SIZE,
                r * D + b * BLOCK_SIZE : r * D + (b + 1) * BLOCK_SIZE,
            ] = blocks[r, b]
    return bdw


def _tiles_per_block(sched):
    return [sum(sg["ntiles"] for sg in blk["sgs"]) for blk in sched]


def _wrap_idxs(src_pad_core, tiles_per_block):
    """Pack per-block gather indices in the dma_gather wrapped layout:
    index j of a block lives at [j % 16, j // 16], replicated across the 8
    groups of 16 partitions. Blocks are concatenated along the free dim.
    Returns [128, Ttot*8] int16."""
    cols = []
    off = 0
    for tb in tiles_per_block:
        ni = int(tb) * TILE_E
        seg = src_pad_core[off : off + ni]
        wrapped = seg.reshape(ni // 16, 16).T
        cols.append(np.tile(wrapped, (8, 1)))
        off += ni
    return np.ascontiguousarray(np.concatenate(cols, axis=1))


# ----------------------------------------------------------------------------
# Bass kernel builder (one SPMD program for all cores)

def _build_nc(sched, Ttot, allow_act_oh=True):
    tiles_per_block = _tiles_per_block(sched)

    nc = bacc.Bacc("TRN2", target_bir_lowering=False, debug=False, num_devices=N_CORES)

    # x table bit-packed as int64 (256B rows); gather cost scales with
    # element count so this is 4x cheaper on Pool than an fp16 view.
    x_d = nc.declare_dram_parameter("xpk", [N_NODES, PACK32], I32, isOutput=False)
    # transposed fp16 x slice for this core's own nodes (self-loop lhsT)
    xt_d = nc.declare_dram_parameter("xt16", [D, NBLK * BLK], F16, isOutput=False)
    srcidx_d = nc.declare_dram_parameter("srcidx", [128, Ttot * 8], I16, isOutput=False)
    metaf_cols = 4 * Ttot  # per block: [tloc | w | -tloc | -w]
    metaf_d = nc.declare_dram_parameter("metaf", [128, metaf_cols], F32, isOutput=False)
    meta16_cols = 512 + NRELS * D
    meta16_d = nc.declare_dram_parameter("meta16", [128, meta16_cols], F16, isOutput=False)
    out_d = nc.declare_dram_parameter("out", [NBLK * BLK, D], F32, isOutput=True)

    with tile.TileContext(nc) as tc:
        with (
            tc.tile_pool(name="const", bufs=1) as const_pool,
            tc.tile_pool(name="sidx", bufs=4) as sidx_pool,
            tc.tile_pool(name="xt", bufs=4) as xt_pool,
            tc.tile_pool(name="xg", bufs=4) as xg_pool,
            tc.tile_pool(name="oh", bufs=3) as oh_pool,
            tc.tile_pool(name="t1", bufs=2) as t1_pool,
            tc.tile_pool(name="aggsb", bufs=6) as aggsb_pool,
            tc.tile_pool(name="outsb", bufs=3) as outsb_pool,
            tc.tile_pool(name="psA", bufs=6, space=bass.MemorySpace.PSUM) as psA_pool,
            tc.tile_pool(name="psO", bufs=2, space=bass.MemorySpace.PSUM) as psO_pool,
        ):
            max_tb = max(tiles_per_block)
            # startup: block 0's gather indices go FIRST on SP so the first
            # gather can begin while metaf/meta16 stream in behind it.
            sidx_tiles = {}
            xt_tiles = {}
            blk_off = [0]
            for bb in range(NBLK):
                blk_off.append(blk_off[-1] + tiles_per_block[bb])
            sidx_tiles[0] = sidx_pool.tile([128, tiles_per_block[0] * 8], I16, tag="sidx", name="sidx")
            nc.sync.dma_start(sidx_tiles[0][:], srcidx_d[:, 0 : tiles_per_block[0] * 8])
            metaf_sb = const_pool.tile([128, metaf_cols], F32, tag="metaf")
            nc.sync.dma_start(
                metaf_sb[:, 0 : 4 * tiles_per_block[0]],
                metaf_d[:, 0 : 4 * tiles_per_block[0]],
            )
            meta16_sb = const_pool.tile([128, meta16_cols], F16, tag="meta16")
            nc.scalar.dma_start(meta16_sb[:, 0:512], meta16_d[:, 0:512])
            nc.sync.dma_start(meta16_sb[:, 512:], meta16_d[:, 512:])
            xt_tiles[0] = xt_pool.tile([128, BLK], F16, tag="xt", name="xt")
            nc.sync.dma_start(xt_tiles[0][:], xt_d[:, 0:BLK])
            iota_sb = meta16_sb[:, 0:512]
            bdw_sb = meta16_sb[:, 512:]
            zeros_sb = const_pool.tile([1, 4 * BLK], F16, tag="zeros")
            nc.vector.memset(zeros_sb[:], 0.0)

            # Adaptive work distribution: track each engine's accumulated
            # busy (updated as gathers/copies/one-hots are emitted, in
            # program order, so it mirrors the runtime timeline).  Below the
            # PE-busy ceiling, pick the CHEAPEST engine for each op (Pool
            # wins narrow one-hots, DVE wide ones); above it, equalize.
            def oh_cost(eng, w):
                if eng == "dve":
                    return 72.0 + 0.263 * w
                if eng == "pool":
                    return 51.0 + 0.8333 * w
                return 120.0 + 2 * (185.0 + 0.8333 * w)

            widths = [
                t["a1"] - t["a0"]
                for bb in range(NBLK)
                for sg in sched[bb]["sgs"]
                for t in sg["tiles"]
            ]
            n_tr = NBLK + sum(len(sg["rels"]) for blk in sched for sg in blk["sgs"])
            pe_est = sum(widths) * 0.4167 + n_tr * 53.3 + 500
            # fraction of run elapsed scales the ceiling so early ops don't
            # all pile onto one engine before the others' fixed work arrives
            # pass 1: plan final per-engine totals with a seeded greedy;
            # pass 2 (emission) redistributes temporally via per-width-class
            # Bresenham so every stretch of the run has the same engine mix.
            useds = [len(sg["rels"]) * BLK for blk in sched for sg in blk["sgs"]]
            acc = {"dve": NBLK * 258.0, "pool": Ttot * TILE_E * 0.4167 + 1100, "act": 1800.0}

            def _wclass(w):
                return 0 if w <= 40 else (1 if w <= 200 else 2)

            # Exact min-max split: tiles sorted by width; Pool takes the P
            # narrowest, ACT the next A, DVE the rest (widest); ACT also
            # takes c_a of the 40 PSUM copies, DVE the others.  Brute-force
            # the three counts with prefix sums (~0.1s on host).
            ws = np.sort(np.asarray(widths, dtype=np.float64))
            n_oh = len(ws)
            pre = {
                e: np.concatenate(
                    [[0.0], np.cumsum([oh_cost(e, w) for w in ws])]
                )
                for e in ("dve", "pool", "act")
            }
            dve_oh_sufsum = pre["dve"][n_oh] - pre["dve"]
            pool_fix, act_fix = acc["pool"], acc["act"]
            n_cp = len(useds)
            best = (1e18, 0, 0, 0)
            for c_a in range(n_cp + 1):
                a_cost = act_fix + 612.0 * c_a
                d_cost = 658.0 * (n_cp - c_a)
                for P in range(0, n_oh + 1, 2):
                    pool_busy = pool_fix + pre["pool"][P]
                    if pool_busy > best[0]:
                        break
                    for A in ((0, 4, 8, 16, 24, 32, 48, 64) if allow_act_oh else (0,)):
                        if P + A > n_oh:
                            break
                        act_busy = a_cost + pre["act"][P + A] - pre["act"][P]
                        dve_busy = d_cost + dve_oh_sufsum[P + A]
                        m = max(pool_busy, act_busy, dve_busy, pe_est)
                        if m < best[0]:
                            best = (m, c_a, P, A)
            _, c_a, P, A = best
            n_copy_act, n_copy_dve = c_a, n_cp - c_a
            thr_pool = ws[P - 1] if P > 0 else -1.0
            thr_act = ws[P + A - 1] if A > 0 else thr_pool
            # per-tile engine: by width thresholds (ties broken by quota)
            quota = {"pool": P, "act": A}
            cls_counts = [{"dve": 0, "pool": 0, "act": 0} for _ in range(3)]
            assigned = {}
            for i in sorted(range(n_oh), key=lambda i: widths[i]):
                if quota["pool"] > 0 and widths[i] <= thr_pool:
                    e = "pool"
                    quota["pool"] -= 1
                elif quota["act"] > 0 and widths[i] <= thr_act:
                    e = "act"
                    quota["act"] -= 1
                else:
                    e = "dve"
                assigned[i] = e
                cls_counts[_wclass(widths[i])][e] += 1
            copy_engines = []
            nv = n_copy_dve
            for k in range(n_cp):
                if nv > 0 and (k % max(1, n_cp // max(n_copy_dve, 1))) == 1:
                    copy_engines.append("dve")
                    nv -= 1
                else:
                    copy_engines.append("act")
            acc = {"dve": best[0], "pool": 0.0, "act": 0.0}
            global LAST_PLAN
            LAST_PLAN = {"cls": cls_counts, "acc": dict(acc),
                         "copies": {e: copy_engines.count(e) for e in set(copy_engines)},
                         "pe_est": pe_est}
            cls_tot = [max(1, sum(c.values())) for c in cls_counts]
            cls_frac = [
                {e: cls_counts[i][e] / cls_tot[i] for e in cls_counts[i]}
                for i in range(3)
            ]
            cls_bres = [{e: 0.0 for e in cls_counts[i]} for i in range(3)]
            copy_i = [0]

            def emit_copy(agg_sb, agg_ps, used):
                e = copy_engines[copy_i[0]]
                copy_i[0] += 1
                if e == "act":
                    nc.scalar.copy(agg_sb[:, :used], agg_ps[:, :used])
                else:
                    nc.vector.tensor_copy(agg_sb[:, :used], agg_ps[:, :used])

            oh_count = [0]

            def emit_oh(out_ap, iota_ap, secs, tl, w):
                tloc_b, w_b, ntloc_b, nw_b = secs
                oh_count[0] += 1
                ci = _wclass(w)
                bres, frac = cls_bres[ci], cls_frac[ci]
                for e in bres:
                    bres[e] += frac[e]
                eng = max(bres, key=lambda e: bres[e])
                bres[eng] -= 1.0
                _emit_oh_on(eng, out_ap, iota_ap, secs, tl, w)

            def _emit_oh_on(eng, out_ap, iota_ap, secs, tl, w):
                tloc_b, w_b, ntloc_b, nw_b = secs
                if eng == "act":
                    t1 = t1_pool.tile([128, 4 * BLK], F16, tag="t1")
                    nc.scalar.activation(
                        t1[:, 0:w], iota_ap, mybir.ActivationFunctionType.Abs,
                        bias=ntloc_b[:, tl : tl + 1], scale=1.0,
                    )
                    nc.scalar.activation(
                        out_ap, t1[:, 0:w], mybir.ActivationFunctionType.Relu,
                        bias=w_b[:, tl : tl + 1],
                        scale=nw_b[:, tl : tl + 1],
                    )
                else:
                    e = nc.vector if eng == "dve" else nc.gpsimd
                    e.tensor_scalar(
                        out_ap, iota_ap,
                        tloc_b[:, tl : tl + 1], w_b[:, tl : tl + 1],
                        mybir.AluOpType.is_equal, mybir.AluOpType.mult,
                    )

            # deferred transform pipeline: transforms of supergroup i run
            # after the scatters of supergroup i+1 (crossing block borders),
            # hiding the PSUM->SBUF copy latency from PE.
            deferred = []  # entries: dict(agg_sb, rels, slots, blkinfo)

            def flush_one():
                if not deferred:
                    return
                ent = deferred.pop(0)
                bi = ent["blkinfo"]
                for r in ent["rels"]:
                    j = ent["slots"][r]
                    bi["ti"] += 1
                    nc.tensor.matmul(
                        bi["out_ps"][:],
                        ent["agg_sb"][:, j * BLK : (j + 1) * BLK],
                        bdw_sb[:, r * D : (r + 1) * D],
                        start=False,
                        stop=(bi["ti"] == bi["n_transforms"]),
                    )
                if bi["ti"] == bi["n_transforms"]:
                    out_sb = outsb_pool.tile([BLK, D], F32, tag="outsb")
                    nc.vector.tensor_copy(out_sb[:], bi["out_ps"][:])
                    b = bi["b"]
                    nc.sync.dma_start(out_d[b * BLK : (b + 1) * BLK, :], out_sb[:])

            tcol = 0       # global tile counter (column into tloc/w)
            for b in range(NBLK):
                tb = tiles_per_block[b]
                if tb == 0:
                    continue
                mo = 4 * blk_off[b]
                if b not in sidx_tiles:
                    sidx_tiles[b] = sidx_pool.tile([128, tb * 8], I16, tag="sidx", name="sidx")
                    nc.sync.dma_start(
                        sidx_tiles[b][:],
                        srcidx_d[:, 8 * blk_off[b] : 8 * blk_off[b] + tb * 8],
                    )
                    nc.sync.dma_start(
                        metaf_sb[:, mo : mo + 4 * tb], metaf_d[:, mo : mo + 4 * tb]
                    )
                    xt_tiles[b] = xt_pool.tile([128, BLK], F16, tag="xt", name="xt")
                    nc.sync.dma_start(xt_tiles[b][:], xt_d[:, b * BLK : (b + 1) * BLK])
                sidx = sidx_tiles[b]
                xt = xt_tiles[b]
                secs = (
                    metaf_sb[:, mo : mo + tb],
                    metaf_sb[:, mo + tb : mo + 2 * tb],
                    metaf_sb[:, mo + 2 * tb : mo + 3 * tb],
                    metaf_sb[:, mo + 3 * tb : mo + 4 * tb],
                )

                # gather all source rows for this block: [e%128, tile, din].
                # Block 0's gather is split so compute starts sooner.
                xg = xg_pool.tile([128, max_tb, PACK32], I32, tag="xg")
                # chunked gathers: Pool interleaves one-hot builds between
                # chunks instead of blocking on one 1.8us transfer, and PE
                # can start as soon as the first tiles land
                splits = [2, 4] if b == 0 else []
                left = tb - sum(splits)
                while left > 0:
                    c_ = min(4, left)
                    splits.append(c_)
                    left -= c_
                off = 0
                for sp in splits:
                    if sp <= 0:
                        continue
                    nc.gpsimd.dma_gather(
                        out_ap=xg[:, off : off + sp, :],
                        in_ap=x_d[:, :],
                        idxs_ap=sidx[:, off * 8 : (off + sp) * 8],
                        num_idxs=sp * TILE_E,
                        num_idxs_reg=sp * TILE_E,
                        elem_size=PACK32,
                        single_packet=False,
                    )
                    off += sp
                xg16 = xg[:, :, :].bitcast(F16)  # [128, max_tb, 128] view

                out_ps = psO_pool.tile([BLK, D], F32, tag="outps")
                blkinfo = {
                    "b": b,
                    "out_ps": out_ps,
                    "n_transforms": 1 + sum(len(sg["rels"]) for sg in sched[b]["sgs"]),
                    "ti": 0,
                }
                gt = 0
                oh_blk = oh_pool.tile([128, max_tb, 4 * BLK], F16, tag="oh")
                bt = 0
                blkinfo["ti"] = 1
                self_emitted = False
                tl = 0  # block-local tile index into metaf sections
                sgs_b = sched[b]["sgs"]
                for sg in sgs_b:
                    agg_ps = psA_pool.tile([D, 4 * BLK], F32, tag="aggps")
                    if sg["need_reset"]:
                        # no merged tile to reset the bank: zero-matmul
                        nc.tensor.matmul(
                            agg_ps[:, 0 : sg["used"]],
                            iota_sb[0:1, 0:D],
                            zeros_sb[0:1, 0 : sg["used"]],
                            start=True,
                            stop=False,
                            skip_group_check=True,
                        )
                    for t in sg["tiles"]:
                        a0, a1 = t["a0"], t["a1"]
                        oh = oh_blk[:, bt, :]
                        emit_oh(oh[:, a0:a1], iota_sb[:, a0:a1], secs, tl, a1 - a0)
                        nc.tensor.matmul(
                            agg_ps[:, a0:a1],
                            xg16[:, gt, :],
                            oh[:, a0:a1],
                            start=t["start"],
                            stop=t["stop"],
                            skip_group_check=True,
                        )
                        tcol += 1
                        tl += 1
                        gt += 1
                        bt += 1
                    if not self_emitted:
                        # self-loop transform opens the block's output PSUM
                        # group; emitted after the first supergroup's
                        # scatters so PE starts on scatter work immediately.
                        self_emitted = True
                        nc.tensor.matmul(
                            out_ps[:],
                            xt[:],
                            bdw_sb[:, NUM_REL * D : (NUM_REL + 1) * D],
                            start=True,
                            stop=(blkinfo["n_transforms"] == 1),
                        )
                    # copy this supergroup's bank to SBUF (ACT/Pool RR)
                    used = len(sg["rels"]) * BLK
                    agg_sb = aggsb_pool.tile([D, 4 * BLK], F16, tag="aggsb")
                    emit_copy(agg_sb, agg_ps, used)
                    # PE: run the previous supergroup's transforms now
                    flush_one()
                    deferred.append(
                        {"agg_sb": agg_sb, "rels": sg["rels"], "slots": sg["slots"],
                         "blkinfo": blkinfo}
                    )
                    if b == NBLK - 1:
                        # drain eagerly at the end to shorten the tail chain
                        flush_one()
            while deferred:
                flush_one()
    nc.compile()
    return nc


# ----------------------------------------------------------------------------

def _make_in_maps(x, node_keep_mask, sched, Ttot, posmaps, src_pad, tloc_pad, w_pad, blocks):
    pos_core, pos_blk, pos_tloc = posmaps
    bdw = _make_bdw(blocks)
    iota512 = np.tile(np.arange(512, dtype=np.float32)[None, :], (128, 1))
    tpb = _tiles_per_block(sched)

    x16 = x.astype(np.float16)
    xpk = np.ascontiguousarray(x16).view(np.int32)  # [N, 64] bit-packed
    # self-loop: dropped-out nodes contribute zero -> mask the transposed copy
    keep = np.asarray(node_keep_mask).astype(np.float16)
    x16m = x16 * keep[:, None]
    meta16 = np.ascontiguousarray(
        np.concatenate([iota512, bdw], axis=1).astype(np.float16)
    )
    in_maps = []
    for c in range(N_CORES):
        tl = tloc_pad[c].reshape(Ttot, 128).T
        wl = w_pad[c].reshape(Ttot, 128).T
        secs = []
        off = 0
        for tb in tpb:  # per block: [tloc | w | -tloc | -w]
            t_b = tl[:, off : off + tb]
            w_b = wl[:, off : off + tb]
            secs += [t_b, w_b, -t_b, -w_b]
            off += tb
        metaf = np.concatenate(secs, axis=1)
        xt = np.zeros((D, NBLK * BLK), dtype=np.float16)
        mine = np.nonzero(pos_core == c)[0]
        xt[:, pos_blk[mine] * BLK + pos_tloc[mine]] = x16m[mine].T
        in_maps.append(
            {
                "xpk": xpk,
                "xt16": np.ascontiguousarray(xt),
                "srcidx": _wrap_idxs(src_pad[c], tpb),
                "metaf": np.ascontiguousarray(metaf),
                "meta16": meta16,
            }
        )
    return in_maps


def kernel(x, node_keep_mask, source, target, edge_type, edge_weights, blocks):
    global LAST_NC, LAST_IN_MAPS
    x = np.ascontiguousarray(np.asarray(x), dtype=np.float32)
    sched, Ttot, posmaps, src_pad, tloc_pad, w_pad = _preprocess(
        source, target, edge_type, edge_weights
    )
    in_maps = _make_in_maps(
        x, node_keep_mask, sched, Ttot, posmaps, src_pad, tloc_pad, w_pad, blocks
    )
    pos_core, pos_blk, pos_tloc = posmaps
    rowsel = pos_blk * BLK + pos_tloc
    # the ACT one-hot path computes Relu(w - w*|iota - tloc|), valid only
    # for non-negative weights (true for this problem's uniform[0,1) fill)
    allow_act_oh = not bool((np.asarray(edge_weights) < 0).any())
    nc = _build_nc(sched, Ttot, allow_act_oh)
    LAST_NC, LAST_IN_MAPS = nc, in_maps

    if _DEBUG_SIM:
        from concourse.bass_interp import CoreSim

        per_core = []
        for c in range(N_CORES):
            sim = CoreSim(nc)
            for k, v in in_maps[c].items():
                sim.tensor(k)[:] = v
            sim.simulate()
            per_core.append(np.array(sim.tensor("out")))
        out = np.empty((N_NODES, D), dtype=np.float32)
        for c in range(N_CORES):
            mine = np.nonzero(pos_core == c)[0]
            out[mine] = per_core[c][rowsel[mine]]
        return out

    trace = os.environ.get("KERNEL_TRACE", "0") == "1"
    res = run_bass_kernel_spmd(
        nc, in_maps, core_ids=list(range(N_CORES)), trace=trace
    )
    global LAST_EXEC_TIME_NS
    LAST_EXEC_TIME_NS = res.exec_time_ns
    out = np.empty((N_NODES, D), dtype=np.float32)
    for c in range(N_CORES):
        mine = np.nonzero(pos_core == c)[0]
        out[mine] = res.results[c]["out"][rowsel[mine]]
    return out


LAST_EXEC_TIME_NS = None
LAST_NC = None
LAST_IN_MAPS = None
LAST_PLAN = None
